# revision 15
# baseline (speedup 1.0000x reference)
"""AdaptiveBoundaryRankingLoss on 8 TRN2 NeuronCores — band algorithm, v2.

loss = (1/K) sum_{pairs} relu(B(|dt|) - (p_hi - p_lo)),
  B(a) = BETA*a/(1+GAMMA*a), K = B(B-1)/2, hi = larger-target index.

Host sorts by PRED ascending. For i > j (dp = p_i - p_j >= 0):
  - discordant pairs (t_i < t_j): contribution = B(|dt|) + dp, relu-free.
    Computed EXACTLY on host in O(n log n) via a weighted merge pass
    (per-i sums of t_j^a over inversions) + the power series of B.
  - concordant pairs (t_i > t_j): relu(B(dt) - dp), nonzero only when
    dp < max B ~ 0.273 -> a narrow band near the diagonal (~5M of 33.5M
    pairs). A global quadratic q(u) ~ B(u) on [0, L] with q(0) <= 0 and
    q concave zeroes discordant band pairs automatically (q(u<0) < 0 <= dp),
    so the band term is relu of a rank-3 bilinear form:
      z_ij = ct_i*t_j + 1*w_j + bias_i*1,
      ct_i = -c1 - 2 c2 t_i,  w_j = c2 t_j^2 + p_j,
      bias_i = c0 + c1 t_i + c2 t_i^2 - p_i.
    The within-block diagonal triangles (z host-computable exactly) are
    folded into the host term.  Plain bf16 everywhere: per-z error
    ~1e-2 worst-case against a 2e-2 relative-error gate on the final
    scalar -> orders of magnitude of margin.

Device (per core, SPMD): the [3,128]x[3,256] chunk tables live on FOUR
7-free 3-partition "lanes" at partition bases 0/32/64/96 (PE row-group
strips), so up to 4 matmuls stream concurrently into 4 DIFFERENT PSUM
banks (concurrent matmuls into the same bank hard-fault).  Chunk k of
lane L goes to PSUM bank seq b=4*(s//2)+L (bank b%8, 256-col half s%2);
2 chunks fill a 512-f32 bank.  ScalarE (Relu activation, accum_out) and
VectorE (tensor_scalar max+add, accum_out) consume whole banks in
statically scheduled contiguous groups; banks 8+ recycle PSUM and wait
on the consuming group's semaphore.  The table is a packed [12, X] bf16
DRAM tensor (only the 12 live partitions are transferred, ~55KB vs the
~590KB a [96,X] layout costs) DMA'd per-lane on the sync HWDGE ring in
an A (stat+first slots) / B (tail slots) split with per-lane semaphores
so the first matmul starts as soon as lane 0's A lands.  A short
dummy-matmul burst keeps PE busy through the DMA ramp.  Per-group
[128,1] partials land in one acc table, DMA'd out once; host reduces in
f64.
"""

import contextlib
import math

import numpy as np
import ml_dtypes

import concourse.bass as bass
from concourse import mybir
from concourse.ap import AP
from concourse.bass_utils import run_bass_kernel_spmd

B = 8192
BETA = 0.3
GAMMA = 0.1
NCORES = 8
P = 128
CH = 256          # matmul chunk width (cols)
CB = CH + P       # per-slot table block: 256 colv + 128 stat cols
NLANES = 4
NBLK = B // P     # 64 row blocks
NDUM = 36         # PE warmup dummy matmuls

_bf16 = ml_dtypes.bfloat16

_NC_CACHE = {}


def _Bfun(a):
    return BETA * a / (1.0 + GAMMA * a)


# ---------- host: exact discordant closed form ----------

def _disc_sums(t, p, M):
    """S[i, a] = sum_{j<i, t_j > t_i} t_j^a (a=0..M); S[i, M+1] same for p_j.
    Bottom-up merge, O(n log n). n must be a power of two."""
    n = len(t)
    W = np.empty((n, M + 2))
    W[:, 0] = 1.0
    for a in range(1, M + 1):
        W[:, a] = W[:, a - 1] * t
    W[:, M + 1] = p
    S = np.zeros((n, M + 2))
    idx = np.arange(n)
    L = 1
    while L < n:
        nruns = n // (2 * L)
        run = idx.reshape(nruns, 2, L)
        li, ri = run[:, 0, :], run[:, 1, :]
        if L <= 64:
            mask = t[li][:, :, None] > t[ri][:, None, :]
            contrib = np.einsum('pji,pjw->piw', mask, W[li])
            S[ri.ravel()] += contrib.reshape(-1, M + 2)
        else:
            for k in range(nruns):
                tl = t[li[k]]
                pos = np.searchsorted(tl, t[ri[k]], side='right')
                suf = np.vstack([np.cumsum(W[li[k]][::-1], axis=0)[::-1],
                                 np.zeros((1, M + 2))])
                S[ri[k]] += suf[pos]
        tv = t[idx].reshape(nruns, 2 * L)
        ordr = np.argsort(tv, axis=1, kind='stable')
        idx = np.take_along_axis(idx.reshape(nruns, 2 * L), ordr, axis=1).ravel()
        L *= 2
    return S


def _disc_closed_form(t, p, M=18):
    """sum over discordant pairs (i>j in p-order, t_j > t_i) of
    B(t_j - t_i) + (p_i - p_j), exact (B via power series)."""
    n = len(t)
    if n & (n - 1) != 0 or (GAMMA * (t.max() - t.min())) > 0.5:
        # fallback: chunked brute force in f64
        tb = 0.0
        for s in range(0, n, 512):
            e = min(s + 512, n)
            u = t[s:e, None] - t[None, :]
            dp = p[s:e, None] - p[None, :]
            lower = (np.arange(s, e)[:, None] > np.arange(n)[None, :])
            disc = lower & (u < 0)
            tb += (_Bfun(-u[disc]) + dp[disc]).sum()
        return tb
    S = _disc_sums(t, p, M)
    total = float((p * S[:, 0]).sum() - S[:, M + 1].sum())
    negt_pow = np.empty((n, M + 1))
    negt_pow[:, 0] = 1.0
    for b in range(1, M + 1):
        negt_pow[:, b] = negt_pow[:, b - 1] * (-t)
    for m in range(1, M + 1):
        Tm = 0.0
        for a in range(0, m + 1):
            Tm += math.comb(m, a) * float((S[:, a] * negt_pow[:, m - a]).sum())
        total += BETA * ((-GAMMA) ** (m - 1)) * Tm
    return total


# ---------- host: quadratic fit of B on [0, L] ----------

def _quad_fit(L):
    x = np.linspace(0.0, L, 8001)
    y = _Bfun(x)
    A = np.stack([np.ones_like(x), x, x * x], 1)
    wts = np.ones_like(x)
    c = np.zeros(3)
    for _ in range(40):
        c = np.linalg.lstsq(A * wts[:, None], y * wts, rcond=None)[0]
        r = np.abs(A @ c - y)
        wts *= (1e-12 + r) ** 0.5
        wts /= wts.max()
    c0, c1, c2 = (float(v) for v in c)
    resid = float(np.abs(c0 + c1 * x + c2 * x * x - y).max())
    if c0 > 0:
        c0 = -1e-6
    assert c1 > 0 and c2 < 0
    return c0, c1, c2, resid


# ---------- static plan ----------

def _relu_sim(runs, engs, NBK):
    """Simulate the relu pipeline for a candidate schedule (HW-fit
    constants).  Returns makespan.  runs: list of (b0, b1); engs: engine
    per run."""
    # per-lane-position DMA readiness (bank pos order) rel. to first data
    RDY = [0.0, 150.0, 510.0, 660.0]
    LBANK = 370.0      # in-lane stream time per bank (2 MMs)
    TES = 420.0        # drain + sem lag from stream end to te_s visible
    PSG = 60.0         # psum-free gate to MM start

    def dur(eng, nb):
        c = nb * 512
        if eng == 'S':
            return (c + 352) / 1.2 + 285.0    # ACTIVATE + READ_ACC
        return (c + 120) / 0.96 + 105.0

    grp_of = {}
    for gi, (b0, b1) in enumerate(runs):
        for b in range(b0, b1 + 1):
            grp_of[b] = gi
    done = [0.0] * NBK       # bank fill (te_s visible)
    gend = [0.0] * len(runs)
    free = {'S': 0.0, 'V': 0.0}
    lane_free = list(RDY)
    prev_issue = 0.0
    for b in range(NBK):
        pos = b % 4
        gate = max(lane_free[pos], prev_issue)
        if b >= 8:
            gate = max(gate, gend[grp_of[b - 8]] + PSG)
        s = gate + LBANK
        lane_free[pos] = s
        prev_issue = gate      # issue-order: next bank can't start earlier
        done[b] = s + TES
        gi = grp_of[b]
        b0, b1 = runs[gi]
        if b == b1:
            e = engs[gi]
            start = max(free[e], max(done[bb] for bb in range(b0, b1 + 1)))
            gend[gi] = start + dur(e, b1 - b0 + 1)
            free[e] = gend[gi]
    return max(gend)


def _relu_plan(NSL):
    """Static schedule: bank-seq -> relu group runs + engine.  Exhaustive
    search (runs <= 3 banks, no 8-bank PSUM-cycle crossing, alternating
    engine assignment searched fully) over a dependency-aware timing model.
    Returns list of (b0, b1, eng) in start order."""
    NBK = 2 * NSL

    def comps(lo, hi):
        if lo == hi:
            return [[]]
        out = []
        for sz in (1, 2, 3):
            if lo + sz <= hi and (lo // 8) == ((lo + sz - 1) // 8):
                for rest in comps(lo + sz, hi):
                    out.append([(lo, lo + sz - 1)] + rest)
        return out

    best = None
    for runs in comps(0, NBK):
        ng = len(runs)
        if ng > 14:
            continue
        for mask in range(1 << ng):
            engs = ['S' if (mask >> i) & 1 else 'V' for i in range(ng)]
            mk = _relu_sim(runs, engs, NBK)
            if best is None or mk < best[0]:
                best = (mk, runs, engs)
    mk, runs, engs = best
    return [(b0, b1, e) for (b0, b1), e in zip(runs, engs)]


# ---------- bass graph ----------

def build_nc(NSL):
    nc = bass.Bass(target_bir_lowering=False, debug=False)
    f32 = mybir.dt.float32
    bf = mybir.dt.bfloat16
    Relu = mybir.ActivationFunctionType.Relu
    A = mybir.AluOpType

    NCH = NLANES * NSL
    NBK = 2 * NSL
    plan = _relu_plan(NSL)
    NG = len(plan)
    NGV = sum(1 for g in plan if g[2] == 'V')
    XC = NSL * CB                  # table cols per partition

    NGS = NG - NGV
    # bank-seq -> (engine, ordinal within engine) for psum-reuse waits;
    # group -> acc column (S groups first, then V groups)
    eng_ord = {}
    colof = {}
    cS = cV = 0
    for g, (b0, b1, eng) in enumerate(plan):
        if eng == 'S':
            colof[g] = cS
            cS += 1
            o = cS
        else:
            colof[g] = NGS + cV
            cV += 1
            o = cV
        for bb in range(b0, b1 + 1):
            eng_ord[bb] = (eng, o)

    tbl_d = nc.declare_dram_parameter("tbl", [3 * NLANES, XC], bf,
                                      isOutput=False)
    out_d = nc.declare_dram_parameter("acc", [P, NG], f32, isOutput=True)

    es = contextlib.ExitStack()
    with es:
        def sb(name, shape, dtype):
            return es.enter_context(nc.sbuf_tensor(name, shape, dtype))

        tbl = sb("tbl_s", [128, XC], bf)
        junk = sb("junk", [3, 64], bf)
        wS = sb("wS", [P, 2048], bf)
        wV = sb("wV", [P, 2048], bf)
        acc = sb("acc_s", [P, NG], f32)
        ps = es.enter_context(nc.psum_tensor("ps", [P, 4096], f32))
        dq = [es.enter_context(nc.semaphore(f"dq{q}")) for q in range(NLANES)]
        te_s = es.enter_context(nc.semaphore("te_s"))
        sS = es.enter_context(nc.semaphore("sS"))
        sV = es.enter_context(nc.semaphore("sV"))

        # SDMA engines are keyed by dst partition: lanes 0,1 ride engine 0,
        # lanes 2,3 ride engine 1.  Issue lanes {0,1} from the sync HWDGE
        # ring and {2,3} from the scalar ring so the ~0.7us per-dma issue
        # cost runs on two sequencers in parallel.  Lane readiness order is
        # then 0,2 first, 1,3 second -> banks pick lanes in that order.
        LMAP = [0, 2, 1, 3]

        def lane_dma(eng, L):
            eng.dma_start(
                out=tbl[32 * L:32 * L + 3, 0:XC],
                in_=tbl_d[3 * L:3 * L + 3, 0:XC],
            ).then_inc(dq[L], 16)

        block = es.enter_context(nc.Block())

        @block.sync
        def _(sync):
            lane_dma(sync, 0)
            lane_dma(sync, 1)
            sync.wait_ge(sS, NGS)
            sync.wait_ge(sV, NGV)
            sync.dma_start(out=out_d[:, :],
                           in_=acc[:, :]).then_inc(dq[0], 16)

        @block.tensor
        def _(tensor):
            # dummy burst: keeps PE busy through the DMA ramp (HAM window)
            for _ in range(NDUM):
                tensor.matmul(ps[0:64, 3584:3648], junk[:, :], junk[:, :],
                              start=True, stop=True)
            tier_seen = [0] * NLANES
            reuse_seen = {'S': 0, 'V': 0}
            for b in range(NBK):
                L = LMAP[b % NLANES]
                r = b // NLANES
                if tier_seen[L] < 1:
                    tensor.wait_ge(dq[L], 16)
                    tier_seen[L] = 1
                if b >= 8:
                    eng, o = eng_ord[b - 8]
                    if reuse_seen[eng] < o:
                        tensor.wait_ge(sS if eng == 'S' else sV, o)
                        reuse_seen[eng] = o
                mm = None
                for half in range(2):
                    s = 2 * r + half
                    base = s * CB
                    mm = tensor.matmul(
                        ps[:, (b % 8) * 512 + half * CH:
                           (b % 8) * 512 + (half + 1) * CH],
                        tbl[32 * L:32 * L + 3, base + CH:base + CB],
                        tbl[32 * L:32 * L + 3, base:base + CH],
                        start=True, stop=True,
                        tile_position=(32 * L, 0),
                    )
                mm.then_inc(te_s, 1)

        @block.scalar
        def _(scalar):
            # dummy activation first: walrus puts ACT_TABLE_LOAD right
            # before it, and the load is async on the engine, so it overlaps
            # the DMA issues that follow
            scalar.activation(wS[:, 0:1], wS[:, 0:1], Relu, bias=0.0)
            lane_dma(scalar, 2)
            lane_dma(scalar, 3)
            for g, (b0, b1, eng) in enumerate(plan):
                if eng != 'S':
                    continue
                cols = (b1 - b0 + 1) * 512
                c = colof[g]
                scalar.wait_ge(te_s, b1 + 1)
                scalar.activation(
                    wS[:, :cols], ps[:, (b0 % 8) * 512:(b0 % 8) * 512 + cols],
                    Relu, bias=0.0, scale=1.0,
                    accum_out=acc[:, c:c + 1],
                ).then_inc(sS, 1)

        @block.vector
        def _(vector):
            for g, (b0, b1, eng) in enumerate(plan):
                if eng != 'V':
                    continue
                cols = (b1 - b0 + 1) * 512
                c = colof[g]
                vector.wait_ge(te_s, b1 + 1)
                vector.tensor_scalar(
                    out=wV[:, :cols],
                    in0=ps[:, (b0 % 8) * 512:(b0 % 8) * 512 + cols],
                    scalar1=0.0, scalar2=0.0, op0=A.max, op1=A.add,
                    accum_out=acc[:, c:c + 1],
                ).then_inc(sV, 1)

    return nc


def _get_nc(NSL):
    if NSL not in _NC_CACHE:
        _NC_CACHE[NSL] = build_nc(NSL)
    return _NC_CACHE[NSL]


# ---------- host: layout + input baking ----------

def _prepare(pred, target):
    p64 = np.asarray(pred, np.float64)
    t64 = np.asarray(target, np.float64)
    n = len(p64)
    order = np.argsort(p64, kind="stable")
    p = p64[order]
    t = t64[order]

    host_total = _disc_closed_form(t, p)

    Lspan = float(t.max() - t.min())
    Lspan = max(Lspan, 1e-6)
    c0, c1, c2, resid = _quad_fit(Lspan)
    qmax = max(_Bfun(Lspan), c0 + c1 * Lspan + c2 * Lspan * Lspan)
    DPMAX = qmax + 2 * resid + 1e-6

    # diagonal 128x128 triangles: exact host relu-sum (z is host-known)
    tb = t.reshape(NBLK, P)
    pb = p.reshape(NBLK, P)
    u = tb[:, :, None] - tb[:, None, :]
    dpd = pb[:, :, None] - pb[:, None, :]
    zd = c0 + c1 * u + c2 * u * u - dpd
    m = np.tril(np.ones((P, P), bool), -1)[None, :, :]
    host_total += float(np.where(m, np.maximum(zd, 0.0), 0.0).sum())

    lo = np.searchsorted(p, p - DPMAX, side="left")

    nch_b = []
    for b in range(NBLK):
        r0 = P * b
        span = r0 - int(lo[r0])
        nch_b.append((span + CH - 1) // CH)

    # greedy balance blocks' chunks over cores, then over 4 lanes per core
    loads = [0] * NCORES
    assign = [[] for _ in range(NCORES)]
    for b in sorted(range(NBLK), key=lambda b: -nch_b[b]):
        c = min(range(NCORES), key=lambda c: loads[c])
        loads[c] += nch_b[b]
        assign[c].append(b)

    # flat chunk list per core; chunk i -> lane i%4, slot i//4.  NSL even so
    # every PSUM bank gets exactly 2 chunk slots.
    core_chunks = []
    for c in range(NCORES):
        chunks = [(b, k) for b in assign[c] for k in range(nch_b[b])]
        core_chunks.append(chunks)
    maxch = max(len(ch) for ch in core_chunks)
    NSL = 2 * ((maxch + 2 * NLANES - 1) // (2 * NLANES))
    lane_chunks = []
    for c in range(NCORES):
        lanes = [[] for _ in range(NLANES)]
        for i, bk in enumerate(core_chunks[c]):
            lanes[i % NLANES].append(bk)
        lane_chunks.append(lanes)

    plan = _relu_plan(NSL)
    NG = len(plan)
    XC = NSL * CB

    # per-row quantities (f64 -> bf16, plain)
    ct = (-c1 - 2.0 * c2 * t).astype(_bf16)
    w = (c2 * t * t + p).astype(_bf16)
    bias = (c0 + c1 * t + c2 * t * t - p).astype(_bf16)
    bt = t.astype(_bf16)

    in_maps = []
    for c in range(NCORES):
        tblp = np.zeros((3 * NLANES, XC), dtype=_bf16)
        for L in range(NLANES):
            for s, (b, k) in enumerate(lane_chunks[c][L]):
                r0 = P * b
                rows = slice(r0, r0 + P)
                cstart = r0 - CH * (k + 1)
                cols = np.arange(cstart, cstart + CH)
                v = cols >= 0
                cc = np.clip(cols, 0, n - 1)
                o = s * CB
                tblp[3 * L + 0, o:o + CH] = np.where(v, bt[cc], _bf16(0.0))
                tblp[3 * L + 1, o:o + CH] = np.where(v, w[cc], _bf16(0.0))
                tblp[3 * L + 2, o:o + CH] = np.where(v, _bf16(1.0), _bf16(0.0))
                tblp[3 * L + 0, o + CH:o + CB] = ct[rows]
                tblp[3 * L + 1, o + CH:o + CB] = _bf16(1.0)
                tblp[3 * L + 2, o + CH:o + CB] = bias[rows]
        in_maps.append({"tbl": tblp})
    return in_maps, host_total, NSL, n


def kernel(pred, target):
    pred = np.asarray(pred, dtype=np.float32)
    target = np.asarray(target, dtype=np.float32)
    in_maps, host_total, NSL, n = _prepare(pred, target)
    nc = _get_nc(NSL)
    res = run_bass_kernel_spmd(nc, in_maps, core_ids=list(range(NCORES)))
    total = host_total
    for r in res.results:
        total += float(np.asarray(r["acc"], np.float64).sum())
    K = n * (n - 1) // 2
    return np.float32(total / K)


# revision 19
# speedup vs baseline: 1.2040x; 1.2040x over previous
"""AdaptiveBoundaryRankingLoss on 8 TRN2 NeuronCores — band algorithm, v7.

loss = (1/K) sum_{pairs} relu(B(|dt|) - (p_hi - p_lo)),
  B(a) = BETA*a/(1+GAMMA*a), K = B(B-1)/2, hi = larger-target index.

Host sorts by PRED ascending. For i > j (dp = p_i - p_j >= 0):
  - discordant pairs (t_i < t_j): contribution = B(|dt|) + dp, relu-free.
    Computed EXACTLY on host in O(n log n) via a weighted merge pass
    (per-i sums of t_j^a over inversions) + the power series of B.
  - concordant pairs (t_i > t_j): relu(B(dt) - dp), nonzero only when
    dp < max B ~ 0.273 -> a narrow band near the diagonal (~5M of 33.5M
    pairs). A global quadratic q(u) ~ B(u) on [0, L] with q(0) <= 0 and
    q concave zeroes discordant band pairs automatically (q(u<0) < 0 <= dp),
    so the band term is relu of a rank-3 bilinear form:
      z_ij = ct_i*t_j + 1*w_j + bias_i*1,
      ct_i = -c1 - 2 c2 t_i,  w_j = c2 t_j^2 + p_j,
      bias_i = c0 + c1 t_i + c2 t_i^2 - p_i.
    The within-block diagonal triangles (z host-computable exactly) are
    folded into the host term.  Plain bf16 everywhere: per-z error ~1e-2
    worst-case against a 2e-2 relative-error gate on the final scalar.

Device (per core, SPMD): [3,128]^T x [3,256] chunk matmuls from FOUR
3-partition "lanes" at partition bases 0/32/64/96 (walrus requires these
exact matmul base partitions), streaming concurrently into 4 different
PSUM banks (concurrent matmuls into the same bank hard-fault).  Lane L
holds 2*NBL chunk slots; its local bank lb pairs slots {2lb, 2lb+1}.
Bank issue order is a greedy earliest-start schedule over per-lane DMA
arrival; PSUM bank = issue_index % 8, banks 8+ recycle and wait on the
consuming relu group's semaphore.  ScalarE (Relu activation, accum_out)
and VectorE (tensor_scalar max+add, accum_out) consume whole banks in
groups chosen by exhaustive search over a HW-fit timing model.  The
packed [12, X] bf16 table (only live partitions; ~55KB vs ~590KB for a
[96, X] layout) moves in per-lane A (slots 0-3) / B (rest) pieces: A's
on the sync HWDGE ring (~0.7-0.9us per dma_start issue slice, so As are
front-loaded), B's on the scalar ring in parallel.  A dummy-matmul burst
keeps PE busy through the DMA ramp; the dummy activation ahead of it
pulls the async ACT_TABLE_LOAD to block start.  The out DMA rides the
otherwise-idle sync ring.  Host reduces the [128, NG] partials in f64.
"""

import contextlib
import math

import numpy as np
import ml_dtypes

import concourse.bass as bass
from concourse import mybir
from concourse.bass_utils import run_bass_kernel_spmd

B = 8192
BETA = 0.3
GAMMA = 0.1
NCORES = 8
P = 128
CH = 256          # matmul chunk width (cols)
CB = CH + P       # per-slot table block: 256 colv + 128 stat cols
NLANES = 4
NBLK = B // P     # 64 row blocks
NDUM = 16         # PE warmup dummy matmuls
A_SLOTS = 4       # slots per lane in the A (sync-ring) DMA piece

_bf16 = ml_dtypes.bfloat16

_NC_CACHE = {}


def _Bfun(a):
    return BETA * a / (1.0 + GAMMA * a)


# ---------- host: exact discordant closed form ----------

def _disc_sums(t, p, M):
    """S[i, a] = sum_{j<i, t_j > t_i} t_j^a (a=0..M); S[i, M+1] same for p_j.
    Bottom-up merge, O(n log n). n must be a power of two."""
    n = len(t)
    W = np.empty((n, M + 2))
    W[:, 0] = 1.0
    for a in range(1, M + 1):
        W[:, a] = W[:, a - 1] * t
    W[:, M + 1] = p
    S = np.zeros((n, M + 2))
    idx = np.arange(n)
    L = 1
    while L < n:
        nruns = n // (2 * L)
        run = idx.reshape(nruns, 2, L)
        li, ri = run[:, 0, :], run[:, 1, :]
        if L <= 64:
            mask = t[li][:, :, None] > t[ri][:, None, :]
            contrib = np.einsum('pji,pjw->piw', mask, W[li])
            S[ri.ravel()] += contrib.reshape(-1, M + 2)
        else:
            for k in range(nruns):
                tl = t[li[k]]
                pos = np.searchsorted(tl, t[ri[k]], side='right')
                suf = np.vstack([np.cumsum(W[li[k]][::-1], axis=0)[::-1],
                                 np.zeros((1, M + 2))])
                S[ri[k]] += suf[pos]
        tv = t[idx].reshape(nruns, 2 * L)
        ordr = np.argsort(tv, axis=1, kind='stable')
        idx = np.take_along_axis(idx.reshape(nruns, 2 * L), ordr, axis=1).ravel()
        L *= 2
    return S


def _disc_closed_form(t, p, M=18):
    """sum over discordant pairs (i>j in p-order, t_j > t_i) of
    B(t_j - t_i) + (p_i - p_j), exact (B via power series)."""
    n = len(t)
    if n & (n - 1) != 0 or (GAMMA * (t.max() - t.min())) > 0.5:
        # fallback: chunked brute force in f64
        tb = 0.0
        for s in range(0, n, 512):
            e = min(s + 512, n)
            u = t[s:e, None] - t[None, :]
            dp = p[s:e, None] - p[None, :]
            lower = (np.arange(s, e)[:, None] > np.arange(n)[None, :])
            disc = lower & (u < 0)
            tb += (_Bfun(-u[disc]) + dp[disc]).sum()
        return tb
    S = _disc_sums(t, p, M)
    total = float((p * S[:, 0]).sum() - S[:, M + 1].sum())
    negt_pow = np.empty((n, M + 1))
    negt_pow[:, 0] = 1.0
    for b in range(1, M + 1):
        negt_pow[:, b] = negt_pow[:, b - 1] * (-t)
    for m in range(1, M + 1):
        Tm = 0.0
        for a in range(0, m + 1):
            Tm += math.comb(m, a) * float((S[:, a] * negt_pow[:, m - a]).sum())
        total += BETA * ((-GAMMA) ** (m - 1)) * Tm
    return total


# ---------- host: quadratic fit of B on [0, L] ----------

def _quad_fit(L):
    x = np.linspace(0.0, L, 8001)
    y = _Bfun(x)
    A = np.stack([np.ones_like(x), x, x * x], 1)
    wts = np.ones_like(x)
    c = np.zeros(3)
    for _ in range(40):
        c = np.linalg.lstsq(A * wts[:, None], y * wts, rcond=None)[0]
        r = np.abs(A @ c - y)
        wts *= (1e-12 + r) ** 0.5
        wts /= wts.max()
    c0, c1, c2 = (float(v) for v in c)
    resid = float(np.abs(c0 + c1 * x + c2 * x * x - y).max())
    if c0 > 0:
        c0 = -1e-6
    assert c1 > 0 and c2 < 0
    return c0, c1, c2, resid


# ---------- static plan ----------

# HW-fit model constants (ns, relative to block entry)
_ARDY = [2400.0, 3100.0, 3800.0, 4500.0]   # lane A-piece data visible
_BRDY = [3700.0, 4500.0, 5200.0, 5900.0]   # lane B-piece data visible
_LBANK = 390.0     # in-lane stream time per bank (2 MMs)
_TES = 420.0       # drain + sem lag from stream end to te_s visible
_PSG = 60.0        # psum-free gate to MM start


def _bank_order(NBL):
    """Issue order of (lane, local-bank) pairs: greedy earliest-start given
    per-lane DMA arrival, in-lane serialization, and the A/B piece split."""
    free = [0.0] * NLANES
    nxt = [0] * NLANES
    order = []
    t_issue = 0.0
    while len(order) < NLANES * NBL:
        cands = []
        for L in range(NLANES):
            if nxt[L] >= NBL:
                continue
            rdy = _ARDY[L] if (2 * nxt[L] + 1) < A_SLOTS else _BRDY[L]
            cands.append((max(free[L], rdy, t_issue), L))
        st, L = min(cands)
        order.append((L, nxt[L]))
        nxt[L] += 1
        t_issue = st
        free[L] = st + _LBANK
    return order


def _relu_sim(runs, engs, BORD):
    """Simulate the relu pipeline for a candidate schedule.  Returns
    makespan.  runs: list of (b0, b1) issue-index ranges; engs: engine per
    run; BORD: bank issue order (lane, lb)."""

    def dur(eng, nb):
        c = nb * 512
        if eng == 'S':
            return (c + 352) / 1.2 + 285.0    # ACTIVATE + READ_ACC
        return (c + 120) / 0.96 + 105.0

    NBK = len(BORD)
    grp_of = {}
    for gi, (b0, b1) in enumerate(runs):
        for b in range(b0, b1 + 1):
            grp_of[b] = gi
    done = [0.0] * NBK
    gend = [0.0] * len(runs)
    free = {'S': 0.0, 'V': 0.0}
    lane_free = [0.0] * NLANES
    prev_issue = 0.0
    for b in range(NBK):
        L, lb = BORD[b]
        rdy = _ARDY[L] if (2 * lb + 1) < A_SLOTS else _BRDY[L]
        gate = max(lane_free[L], rdy, prev_issue)
        if b >= 8:
            gate = max(gate, gend[grp_of[b - 8]] + _PSG)
        e0 = gate + _LBANK
        lane_free[L] = e0
        prev_issue = gate
        done[b] = e0 + _TES
        gi = grp_of[b]
        b0, b1 = runs[gi]
        if b == b1:
            e = engs[gi]
            start = max(free[e], max(done[bb] for bb in range(b0, b1 + 1)))
            gend[gi] = start + dur(e, b1 - b0 + 1)
            free[e] = gend[gi]
    return max(gend)


def _relu_plan(NBL):
    """Static schedule: issue-index -> relu group runs + engine.  Exhaustive
    search (runs <= 3 banks, no 8-bank PSUM-cycle crossing, full engine
    assignment) over the dependency-aware timing model.  Returns
    (bank_order, [(b0, b1, eng)...])."""
    BORD = _bank_order(NBL)
    NBK = len(BORD)

    def comps(lo, hi):
        if lo == hi:
            return [[]]
        out = []
        for sz in (1, 2, 3):
            if lo + sz <= hi and (lo // 8) == ((lo + sz - 1) // 8):
                for rest in comps(lo + sz, hi):
                    out.append([(lo, lo + sz - 1)] + rest)
        return out

    best = None
    for runs in comps(0, NBK):
        ng = len(runs)
        if ng > 14:
            continue
        for mask in range(1 << ng):
            engs = ['S' if (mask >> i) & 1 else 'V' for i in range(ng)]
            mk = _relu_sim(runs, engs, BORD)
            if best is None or mk < best[0]:
                best = (mk, runs, engs)
    mk, runs, engs = best
    return BORD, [(b0, b1, e) for (b0, b1), e in zip(runs, engs)]


# ---------- bass graph ----------

def build_nc(NBL):
    nc = bass.Bass(target_bir_lowering=False, debug=False)
    f32 = mybir.dt.float32
    bf = mybir.dt.bfloat16
    Relu = mybir.ActivationFunctionType.Relu
    A = mybir.AluOpType

    NSL = 2 * NBL                  # chunk slots per lane
    BORD, plan = _relu_plan(NBL)
    NBK = len(BORD)
    NG = len(plan)
    NGV = sum(1 for g in plan if g[2] == 'V')
    NGS = NG - NGV
    XC = NSL * CB                  # table cols per partition
    ACOLS = min(A_SLOTS, NSL) * CB

    # issue-index -> (engine, ordinal within engine) for psum-reuse waits;
    # group -> acc column (S groups first, then V groups)
    eng_ord = {}
    colof = {}
    cS = cV = 0
    for g, (g0, g1, eng) in enumerate(plan):
        if eng == 'S':
            colof[g] = cS
            cS += 1
            o = cS
        else:
            colof[g] = NGS + cV
            cV += 1
            o = cV
        for bb in range(g0, g1 + 1):
            eng_ord[bb] = (eng, o)

    tbl_d = nc.declare_dram_parameter("tbl", [3 * NLANES, XC], bf,
                                      isOutput=False)
    out_d = nc.declare_dram_parameter("acc", [P, NG], f32, isOutput=True)

    es = contextlib.ExitStack()
    with es:
        def sb(name, shape, dtype):
            return es.enter_context(nc.sbuf_tensor(name, shape, dtype))

        tbl = sb("tbl_s", [128, XC], bf)
        junk = sb("junk", [3, 64], bf)
        wS = sb("wS", [P, 2048], bf)
        wV = sb("wV", [P, 2048], bf)
        acc = sb("acc_s", [P, NG], f32)
        ps = es.enter_context(nc.psum_tensor("ps", [P, 4096], f32))
        dq = [es.enter_context(nc.semaphore(f"dq{q}")) for q in range(NLANES)]
        te_s = es.enter_context(nc.semaphore("te_s"))
        sS = es.enter_context(nc.semaphore("sS"))
        sV = es.enter_context(nc.semaphore("sV"))

        def lane_dma(eng, L, c0, c1):
            eng.dma_start(
                out=tbl[32 * L:32 * L + 3, c0:c1],
                in_=tbl_d[3 * L:3 * L + 3, c0:c1],
            ).then_inc(dq[L], 16)

        block = es.enter_context(nc.Block())

        @block.sync
        def _(sync):
            for L in range(NLANES):
                lane_dma(sync, L, 0, ACOLS)
            sync.wait_ge(sS, NGS)
            sync.wait_ge(sV, NGV)
            sync.dma_start(out=out_d[:, :],
                           in_=acc[:, :]).then_inc(dq[0], 16)

        @block.tensor
        def _(tensor):
            # dummy burst: keeps PE busy through the DMA ramp (HAM window)
            for _ in range(NDUM):
                tensor.matmul(ps[0:64, 3584:3648], junk[:, :], junk[:, :],
                              start=True, stop=True)
            tier_seen = [0] * NLANES
            reuse_seen = {'S': 0, 'V': 0}
            for b in range(NBK):
                L, lb = BORD[b]
                tier = 1 if (2 * lb + 1) < A_SLOTS or ACOLS >= XC else 2
                if tier_seen[L] < tier:
                    tensor.wait_ge(dq[L], 16 * tier)
                    tier_seen[L] = tier
                if b >= 8:
                    eng, o = eng_ord[b - 8]
                    if reuse_seen[eng] < o:
                        tensor.wait_ge(sS if eng == 'S' else sV, o)
                        reuse_seen[eng] = o
                mm = None
                for half in range(2):
                    s = 2 * lb + half
                    base = s * CB
                    mm = tensor.matmul(
                        ps[:, (b % 8) * 512 + half * CH:
                           (b % 8) * 512 + (half + 1) * CH],
                        tbl[32 * L:32 * L + 3, base + CH:base + CB],
                        tbl[32 * L:32 * L + 3, base:base + CH],
                        start=True, stop=True,
                        tile_position=(32 * L, 0),
                    )
                mm.then_inc(te_s, 1)

        @block.scalar
        def _(scalar):
            # dummy activation first: walrus puts the async ACT_TABLE_LOAD
            # right before it, overlapping the DMA window
            scalar.activation(wS[:, 0:1], wS[:, 0:1], Relu, bias=0.0)
            if ACOLS < XC:
                for L in range(NLANES):
                    lane_dma(scalar, L, ACOLS, XC)
            for g, (g0, g1, eng) in enumerate(plan):
                if eng != 'S':
                    continue
                cols = (g1 - g0 + 1) * 512
                c = colof[g]
                scalar.wait_ge(te_s, g1 + 1)
                scalar.activation(
                    wS[:, :cols], ps[:, (g0 % 8) * 512:(g0 % 8) * 512 + cols],
                    Relu, bias=0.0, scale=1.0,
                    accum_out=acc[:, c:c + 1],
                ).then_inc(sS, 1)

        @block.vector
        def _(vector):
            for g, (g0, g1, eng) in enumerate(plan):
                if eng != 'V':
                    continue
                cols = (g1 - g0 + 1) * 512
                c = colof[g]
                vector.wait_ge(te_s, g1 + 1)
                vector.tensor_scalar(
                    out=wV[:, :cols],
                    in0=ps[:, (g0 % 8) * 512:(g0 % 8) * 512 + cols],
                    scalar1=0.0, scalar2=0.0, op0=A.max, op1=A.add,
                    accum_out=acc[:, c:c + 1],
                ).then_inc(sV, 1)

    return nc


def _get_nc(NBL):
    if NBL not in _NC_CACHE:
        _NC_CACHE[NBL] = build_nc(NBL)
    return _NC_CACHE[NBL]


# ---------- host: layout + input baking ----------

def _prepare(pred, target):
    p64 = np.asarray(pred, np.float64)
    t64 = np.asarray(target, np.float64)
    n = len(p64)
    order = np.argsort(p64, kind="stable")
    p = p64[order]
    t = t64[order]

    host_total = _disc_closed_form(t, p)

    Lspan = float(t.max() - t.min())
    Lspan = max(Lspan, 1e-6)
    c0, c1, c2, resid = _quad_fit(Lspan)
    qmax = max(_Bfun(Lspan), c0 + c1 * Lspan + c2 * Lspan * Lspan)
    DPMAX = qmax + 2 * resid + 1e-6

    # diagonal 128x128 triangles: exact host relu-sum (z is host-known)
    tb = t.reshape(NBLK, P)
    pb = p.reshape(NBLK, P)
    u = tb[:, :, None] - tb[:, None, :]
    dpd = pb[:, :, None] - pb[:, None, :]
    zd = c0 + c1 * u + c2 * u * u - dpd
    m = np.tril(np.ones((P, P), bool), -1)[None, :, :]
    host_total += float(np.where(m, np.maximum(zd, 0.0), 0.0).sum())

    lo = np.searchsorted(p, p - DPMAX, side="left")

    nch_b = []
    for b in range(NBLK):
        r0 = P * b
        span = r0 - int(lo[r0])
        nch_b.append((span + CH - 1) // CH)

    # greedy balance blocks' chunks over cores
    loads = [0] * NCORES
    assign = [[] for _ in range(NCORES)]
    for b in sorted(range(NBLK), key=lambda b: -nch_b[b]):
        c = min(range(NCORES), key=lambda c: loads[c])
        loads[c] += nch_b[b]
        assign[c].append(b)

    core_chunks = []
    for c in range(NCORES):
        chunks = [(b, k) for b in assign[c] for k in range(nch_b[b])]
        core_chunks.append(chunks)
    maxch = max(len(ch) for ch in core_chunks)
    NBL = (maxch + 2 * NLANES - 1) // (2 * NLANES)
    NSL = 2 * NBL
    XC = NSL * CB

    # chunk i fills the slot consumed i-th by the bank issue order, so
    # cores with fewer chunks leave the latest-consumed slots zero
    BORD, _pl = _relu_plan(NBL)
    slot_seq = []
    for (L, lb) in BORD:
        slot_seq.append((L, 2 * lb))
        slot_seq.append((L, 2 * lb + 1))

    # per-row quantities (f64 -> bf16, plain)
    ct = (-c1 - 2.0 * c2 * t).astype(_bf16)
    w = (c2 * t * t + p).astype(_bf16)
    bias = (c0 + c1 * t + c2 * t * t - p).astype(_bf16)
    bt = t.astype(_bf16)

    in_maps = []
    for c in range(NCORES):
        tblp = np.zeros((3 * NLANES, XC), dtype=_bf16)
        for i, (b, k) in enumerate(core_chunks[c]):
            L, s = slot_seq[i]
            r0 = P * b
            rows = slice(r0, r0 + P)
            cstart = r0 - CH * (k + 1)
            cols = np.arange(cstart, cstart + CH)
            v = cols >= 0
            cc = np.clip(cols, 0, n - 1)
            o = s * CB
            tblp[3 * L + 0, o:o + CH] = np.where(v, bt[cc], _bf16(0.0))
            tblp[3 * L + 1, o:o + CH] = np.where(v, w[cc], _bf16(0.0))
            tblp[3 * L + 2, o:o + CH] = np.where(v, _bf16(1.0), _bf16(0.0))
            tblp[3 * L + 0, o + CH:o + CB] = ct[rows]
            tblp[3 * L + 1, o + CH:o + CB] = _bf16(1.0)
            tblp[3 * L + 2, o + CH:o + CB] = bias[rows]
        in_maps.append({"tbl": tblp})
    return in_maps, host_total, NBL, n


def kernel(pred, target):
    pred = np.asarray(pred, dtype=np.float32)
    target = np.asarray(target, dtype=np.float32)
    in_maps, host_total, NBL, n = _prepare(pred, target)
    nc = _get_nc(NBL)
    res = run_bass_kernel_spmd(nc, in_maps, core_ids=list(range(NCORES)))
    total = host_total
    for r in res.results:
        total += float(np.asarray(r["acc"], np.float64).sum())
    K = n * (n - 1) // 2
    return np.float32(total / K)
